# revision 14
# baseline (speedup 1.0000x reference)
"""HRA (Householder Reflection Adaptation) forward kernel for Trainium2.

Math: out = x @ Q with Q = prod_i (I - 2 u_i u_i^T), u_i = normalized columns
of hra_u [4096, 8].  Using the compact WY representation:
    Q = I - U T U^T      (T upper-triangular 8x8, diag=2)
    out = x - (x @ A) @ U^T,   A = U @ T
so the device only does two skinny matmuls per tile plus a subtract.

Sharding: data-parallel over rows. x [4,2048,4096] -> [8192, 4096]; each of
8 cores gets 1024 contiguous rows. A and U^T are tiny and replicated.

Everything runs in bf16 (inputs quantized host-side; |err| ~ 5e-3 rel, well
inside the 2e-2 gate): HBM traffic halves (16.8 MB/core, ~47 us roofline)
and PE transposes run at 1 cycle/row (vs 2 for f32).

Per-core structure (4 blocks x 256 rows, software-pipelined):
  front(b): per 8-chunk super-group: 16 PE transposes -> 2-bank bf16 PSUM
    strip, one ACT copy (read as f32 pairs to halve the element count)
    -> SBUF x^T; the proj matmuls P^T[8,256] += A_k^T xT_k run one
    super-group behind so the copy latency is hidden.
  back(b-1), interleaved 2 units per front yield: per-(j,c) [128,512]
    update matmul; the PSUM drain is a direct DVE subtract for most units
    and ACT-copy + cheap all-bf16 DVE subtract for a few, so the drain
    spreads across both engines; the last block alternates so the tail
    drains in parallel.
  warm-up: ~4us of matmuls during the initial DMA fill opens the PE HAM
    clock-gate before the first real block.
  output DMAs issue from GPSIMD (SWDGE, free descriptor generation); the
  last block streams out in quarter pieces to shrink the tail.
"""

import os
import sys

for _p in ("/opt/trn_rl_repo", "/root/.axon_site", "/root/.axon_site/_ro/trn_rl_repo",
           "/root/.axon_site/_ro/pypackages"):
    if os.path.isdir(_p) and _p not in sys.path:
        sys.path.append(_p)

import numpy as np
import ml_dtypes

import concourse.bass as bass
import concourse.mybir as mybir
import concourse.tile as tile
from concourse import bacc
from concourse.bass_utils import run_bass_kernel_spmd

B, S, D, R = 4, 2048, 4096, 8
N_CORES = 8
ROWS = B * S                      # 8192
ROWS_PER_CORE = ROWS // N_CORES   # 1024
P = 128
D_CHUNKS = D // P                 # 32
SG = 8                            # d-chunks per transpose super-group
N_SG = D_CHUNKS // SG             # 4 super-groups per block
UPD_CHUNKS = D // 512             # 8 update units per j

F32 = mybir.dt.float32
BF16 = mybir.dt.bfloat16
NP_BF16 = ml_dtypes.bfloat16

_CACHE = {}


def _householder_wy(hra_u):
    """Return (A, UT) with out = x - (x @ A) @ UT."""
    u = hra_u.astype(np.float32)
    u = u / np.linalg.norm(u, axis=0, keepdims=True)
    U = u.astype(np.float64)
    T = np.zeros((R, R), np.float64)
    for k in range(R):
        T[k, k] = 2.0
        if k:
            T[:k, k] = -2.0 * (T[:k, :k] @ (U[:, :k].T @ U[:, k]))
    A = (U @ T).astype(np.float32)          # [D, R]
    return A, np.ascontiguousarray(u.T)     # [R, D]


J = 2                             # 128-row tiles per block
BLK = J * P                       # 256 rows per block
N_BLKS = ROWS_PER_CORE // BLK     # 4 blocks per core

# back-units (j*8+c) whose PSUM drain goes ACT-copy + bf16 DVE sub
# instead of a direct DVE PSUM subtract; the last block alternates so the
# tail drains on both engines in parallel
OFFLOAD = {2, 6, 10, 14}
OFFLOAD_LAST = {1, 3, 5, 7, 9, 11, 13, 15}


def _build_program():
    nc = bacc.Bacc(trn_type="TRN2")
    x = nc.dram_tensor("x", (ROWS_PER_CORE, D), BF16, kind="ExternalInput")
    a = nc.dram_tensor("a", (P, D_CHUNKS * R), BF16, kind="ExternalInput")
    ut = nc.dram_tensor("ut", (R, D), BF16, kind="ExternalInput")
    ident = nc.dram_tensor("ident", (P, P), BF16, kind="ExternalInput")
    out = nc.dram_tensor("out", (ROWS_PER_CORE, D), BF16, kind="ExternalOutput")

    xd = x.rearrange("(b j p) d -> b p j d", p=P, j=J)
    od = out.rearrange("(b j p) d -> b p j d", p=P, j=J)

    with tile.TileContext(nc) as tc:
        with (
            tc.tile_pool(name="const", bufs=1) as const,
            tc.tile_pool(name="xp", bufs=4) as x_pool,
            tc.tile_pool(name="xtp", bufs=2) as xt_pool,
            tc.tile_pool(name="tmp", bufs=2) as tmp_pool,
            tc.tile_pool(name="ptp", bufs=2) as pt_pool,
            tc.tile_pool(name="pst", bufs=2, space="PSUM") as pst_pool,
            tc.tile_pool(name="psp", bufs=1, space="PSUM") as psp_pool,
            tc.tile_pool(name="pso", bufs=3, space="PSUM") as pso_pool,
        ):
            # ident first: the warm-up burst depends only on it
            ident_sb = const.tile([P, P], BF16)
            nc.sync.dma_start(ident_sb, ident[:, :])

            # block-0 leading halves next, so the first transposes start
            # as soon as the leading chunks land
            xbs = []
            xb0 = x_pool.tile([P, J, D], BF16, tag="xb")
            xbs.append(xb0)
            h = D // 2
            for j in range(J):
                nc.sync.dma_start(xb0[:, j, :h], xd[0, :, j, :h])

            a_sb = const.tile([P, D_CHUNKS * R], BF16)
            nc.sync.dma_start(a_sb, a[:, :])
            ut_sb = const.tile([R, D], BF16)
            nc.sync.dma_start(ut_sb, ut[:, :])

            for j in range(J):
                nc.sync.dma_start(xb0[:, j, h:], xd[0, :, j, h:])

            # Prime PE on each constant (one sync-wait per LDWEIGHTS), then
            # ~4us of matmuls during the DMA fill to open the HAM clock-gate.
            warm_t = pst_pool.tile([P, SG, BLK], BF16, name="ps_t", tag="ps_t")
            nc.tensor.transpose(warm_t[:, 0, :P], ident_sb, ident_sb)
            warm = pso_pool.tile([P, 512], F32, tag="ps_o")
            nc.tensor.matmul(warm[:R, :P], a_sb[:, :R], a_sb[:, :P],
                             start=True, stop=True)
            for _ in range(10):
                nc.tensor.matmul(warm[:, :512], ut_sb[:, :P], ut_sb[:, :512],
                                 start=True, stop=True)

            # prefetch the remaining block inputs (one 2 MB DMA each)
            for b in range(1, N_BLKS):
                xb = x_pool.tile([P, J, D], BF16, tag="xb")
                xbs.append(xb)
                nc.sync.dma_start(xb, xd[b])

            def back_units(b, pt):
                """yield per-(j,c) update+subtract callables; the final
                block streams its output in quarter pieces to cut the tail"""
                xb = xbs[b]
                last = b == N_BLKS - 1
                off = OFFLOAD_LAST if last else OFFLOAD

                def unit(j, c):
                    ps_o = pso_pool.tile([P, 512], F32, tag="ps_o")
                    nc.tensor.matmul(
                        ps_o,
                        pt[:, j * P:(j + 1) * P],
                        ut_sb[:, c * 512:(c + 1) * 512],
                        start=True,
                        stop=True,
                    )
                    dst = xb[:, j, c * 512:(c + 1) * 512]
                    if (j * UPD_CHUNKS + c) in off:
                        t = tmp_pool.tile([P, 512], BF16, tag="tmp")
                        nc.scalar.copy(t, ps_o)
                        nc.vector.tensor_sub(dst, dst, t)
                    else:
                        nc.vector.tensor_sub(dst, dst, ps_o)
                    if last:
                        if (c + 1) % 2 == 0:
                            lo, hi = (c - 1) * 512, (c + 1) * 512
                            nc.gpsimd.dma_start(od[b, :, j, lo:hi],
                                                xb[:, j, lo:hi])
                    elif c == UPD_CHUNKS - 1 and j == J - 1:
                        nc.gpsimd.dma_start(od[b], xb)

                for j in range(J):
                    for c in range(UPD_CHUNKS):
                        yield lambda j=j, c=c: unit(j, c)

            def front_units(b):
                """Super-group pipeline: 16 transposes land a [128,8,256]
                bf16 PSUM strip, one ACT copy (read as f32 pairs) moves it
                to SBUF, and the 8 proj matmuls of the PREVIOUS super-group
                run under that copy's latency.  pt lands in pts[b]."""
                ps_p = psp_pool.tile([R, BLK], F32, tag="ps_p")
                xts = [None] * N_SG

                def transposes(sg):
                    ps_t = pst_pool.tile([P, SG, BLK], BF16,
                                         name="ps_t", tag="ps_t")
                    for i in range(SG):
                        k = SG * sg + i
                        for j in range(J):
                            nc.tensor.transpose(
                                ps_t[:, i, j * P:(j + 1) * P],
                                xbs[b][:, j, k * P:(k + 1) * P],
                                ident_sb,
                            )
                    xt_g = xt_pool.tile([P, SG, BLK], BF16, tag="xt_g")
                    nc.scalar.copy(xt_g.bitcast(F32), ps_t.bitcast(F32))
                    xts[sg] = xt_g

                def proj(sg):
                    for i in range(SG):
                        k = SG * sg + i
                        nc.tensor.matmul(
                            ps_p,
                            a_sb[:, k * R:(k + 1) * R],
                            xts[sg][:, i],
                            start=(k == 0),
                            stop=(k == D_CHUNKS - 1),
                        )

                def finish():
                    pt = pt_pool.tile([R, BLK], BF16, tag="pt")
                    nc.vector.tensor_copy(pt, ps_p)
                    pts[b] = pt

                yield lambda: transposes(0)
                for sg in range(1, N_SG):
                    yield lambda sg=sg: transposes(sg)
                    yield lambda sg=sg: proj(sg - 1)
                yield lambda: proj(N_SG - 1)
                yield lambda: finish()

            def drain(it):
                for f in it:
                    f()

            pts = {}
            drain(front_units(0))
            for b in range(1, N_BLKS):
                fu = list(front_units(b))       # 9 units
                bu = list(back_units(b - 1, pts[b - 1]))  # 16 units
                # interleave: 2 back units after each front unit until spent
                order = []
                bi = 0
                for f in fu:
                    order.append(f)
                    for _ in range(2):
                        if bi < len(bu):
                            order.append(bu[bi])
                            bi += 1
                while bi < len(bu):
                    order.append(bu[bi])
                    bi += 1
                drain(order)
            drain(back_units(N_BLKS - 1, pts[N_BLKS - 1]))

    nc.compile()
    return nc


def _get_program():
    if "nc" not in _CACHE:
        _CACHE["nc"] = _build_program()
    return _CACHE["nc"]


def kernel(input, hra_u, **run_kwargs):
    input = np.asarray(input, dtype=np.float32)
    hra_u = np.asarray(hra_u, dtype=np.float32)

    A, UT = _householder_wy(hra_u)
    # pack A [D, R] so partition p holds A[c*128+p, :] at free offset c*R
    a_packed = np.ascontiguousarray(
        A.reshape(D_CHUNKS, P, R).transpose(1, 0, 2).reshape(P, D_CHUNKS * R)
    ).astype(NP_BF16)
    ut_b = UT.astype(NP_BF16)
    ident = np.eye(P, dtype=np.float32).astype(NP_BF16)

    x_flat = np.ascontiguousarray(input.reshape(ROWS, D)).astype(NP_BF16)
    in_maps = [
        {
            "x": x_flat[c * ROWS_PER_CORE:(c + 1) * ROWS_PER_CORE],
            "a": a_packed,
            "ut": ut_b,
            "ident": ident,
        }
        for c in range(N_CORES)
    ]

    nc = _get_program()
    res = run_bass_kernel_spmd(nc, in_maps, core_ids=list(range(N_CORES)),
                               **run_kwargs)
    out = np.concatenate([r["out"] for r in res.results], axis=0)
    if run_kwargs:
        kernel.last_results = res
    return out.astype(np.float32).reshape(B, S, D)


# revision 15
# speedup vs baseline: 1.0680x; 1.0680x over previous
"""HRA (Householder Reflection Adaptation) forward kernel for Trainium2.

Math: out = x @ Q with Q = prod_i (I - 2 u_i u_i^T), u_i = normalized columns
of hra_u [4096, 8].  Using the compact WY representation:
    Q = I - U T U^T      (T upper-triangular 8x8, diag=2)
    out = x - (x @ A) @ U^T,   A = U @ T
so the device only does two skinny matmuls per tile plus a subtract.

Sharding: data-parallel over rows. x [4,2048,4096] -> [8192, 4096]; each of
8 cores gets 1024 contiguous rows. A and U^T are tiny and replicated.

Everything runs in bf16 (inputs quantized host-side; |err| ~ 5e-3 rel, well
inside the 2e-2 gate): HBM traffic halves (16.8 MB/core, ~47 us roofline)
and PE transposes run at 1 cycle/row (vs 2 for f32).

Per-core structure (4 blocks x 256 rows, software-pipelined):
  front(b): per 8-chunk super-group: 16 PE transposes -> 2-bank bf16 PSUM
    strip, one ACT copy (read as f32 pairs to halve the element count)
    -> SBUF x^T; the proj matmuls P^T[8,256] += A_k^T xT_k run one
    super-group behind so the copy latency is hidden.
  back(b-1), interleaved 2 units per front yield: per-(j,c) [128,512]
    update matmul; the PSUM drain is a direct DVE subtract for most units
    and ACT-copy + cheap all-bf16 DVE subtract for a few, so the drain
    spreads across both engines; the last block alternates so the tail
    drains in parallel.
  warm-up: ~4us of matmuls during the initial DMA fill opens the PE HAM
    clock-gate before the first real block.
  output DMAs issue from GPSIMD (SWDGE, free descriptor generation); the
  last block streams out in quarter pieces to shrink the tail.
"""

import os
import sys

for _p in ("/opt/trn_rl_repo", "/root/.axon_site", "/root/.axon_site/_ro/trn_rl_repo",
           "/root/.axon_site/_ro/pypackages"):
    if os.path.isdir(_p) and _p not in sys.path:
        sys.path.append(_p)

import numpy as np
import ml_dtypes

import concourse.bass as bass
import concourse.mybir as mybir
import concourse.tile as tile
from concourse import bacc
from concourse.bass_utils import run_bass_kernel_spmd

B, S, D, R = 4, 2048, 4096, 8
N_CORES = 8
ROWS = B * S                      # 8192
ROWS_PER_CORE = ROWS // N_CORES   # 1024
P = 128
D_CHUNKS = D // P                 # 32
SG = 8                            # d-chunks per transpose super-group
N_SG = D_CHUNKS // SG             # 4 super-groups per block
UPD_CHUNKS = D // 512             # 8 update units per j

F32 = mybir.dt.float32
BF16 = mybir.dt.bfloat16
NP_BF16 = ml_dtypes.bfloat16

_CACHE = {}


def _householder_wy(hra_u):
    """Return (A, UT) with out = x - (x @ A) @ UT."""
    u = hra_u.astype(np.float32)
    u = u / np.linalg.norm(u, axis=0, keepdims=True)
    U = u.astype(np.float64)
    T = np.zeros((R, R), np.float64)
    for k in range(R):
        T[k, k] = 2.0
        if k:
            T[:k, k] = -2.0 * (T[:k, :k] @ (U[:, :k].T @ U[:, k]))
    A = (U @ T).astype(np.float32)          # [D, R]
    return A, np.ascontiguousarray(u.T)     # [R, D]


J = 2                             # 128-row tiles per block
BLK = J * P                       # 256 rows per block
N_BLKS = ROWS_PER_CORE // BLK     # 4 blocks per core

# drain policy per back unit (j*8+c): D = direct DVE PSUM subtract;
# AG = ACT copy + GPSIMD bf16 subtract (GPSIMD is otherwise idle);
# AP1/AP2 = ACT-copy pair - two adjacent units share one [128,1024] bf16
# staging tile and ONE DVE subtract, halving DVE's per-op pipeline-drain
# tax on the offloaded units.  The last block spreads the tail across all
# three engines.
_D, _AG, _AP1, _AP2 = 0, 1, 2, 3
_POL = [_D, _D, _AG, _D, _D, _AG, _AP1, _AP2,
        _D, _D, _AG, _D, _D, _AG, _AP1, _AP2]
_POL_LAST = [_D, _AG, _AP1, _AP2, _D, _AG, _AP1, _AP2,
             _D, _AG, _AP1, _AP2, _D, _AG, _AP1, _AP2]


def _build_program():
    nc = bacc.Bacc(trn_type="TRN2")
    x = nc.dram_tensor("x", (ROWS_PER_CORE, D), BF16, kind="ExternalInput")
    a = nc.dram_tensor("a", (P, D_CHUNKS * R), BF16, kind="ExternalInput")
    ut = nc.dram_tensor("ut", (R, D), BF16, kind="ExternalInput")
    ident = nc.dram_tensor("ident", (P, P), BF16, kind="ExternalInput")
    out = nc.dram_tensor("out", (ROWS_PER_CORE, D), BF16, kind="ExternalOutput")

    xd = x.rearrange("(b j p) d -> b p j d", p=P, j=J)
    od = out.rearrange("(b j p) d -> b p j d", p=P, j=J)

    with tile.TileContext(nc) as tc:
        with (
            tc.tile_pool(name="const", bufs=1) as const,
            tc.tile_pool(name="xp", bufs=4) as x_pool,
            tc.tile_pool(name="xtp", bufs=2) as xt_pool,
            tc.tile_pool(name="tmp", bufs=2) as tmp_pool,
            tc.tile_pool(name="ptp", bufs=2) as pt_pool,
            tc.tile_pool(name="pst", bufs=2, space="PSUM") as pst_pool,
            tc.tile_pool(name="psp", bufs=1, space="PSUM") as psp_pool,
            tc.tile_pool(name="pso", bufs=3, space="PSUM") as pso_pool,
        ):
            # ident first: the warm-up burst depends only on it
            ident_sb = const.tile([P, P], BF16)
            nc.sync.dma_start(ident_sb, ident[:, :])

            # block-0 leading halves next, so the first transposes start
            # as soon as the leading chunks land
            xbs = []
            xb0 = x_pool.tile([P, J, D], BF16, tag="xb")
            xbs.append(xb0)
            h = D // 2
            for j in range(J):
                nc.sync.dma_start(xb0[:, j, :h], xd[0, :, j, :h])

            a_sb = const.tile([P, D_CHUNKS * R], BF16)
            nc.sync.dma_start(a_sb, a[:, :])
            ut_sb = const.tile([R, D], BF16)
            nc.sync.dma_start(ut_sb, ut[:, :])

            for j in range(J):
                nc.sync.dma_start(xb0[:, j, h:], xd[0, :, j, h:])

            # ~4us of ident-only matmuls (gated only on the tiny first
            # DMA) open the HAM clock-gate while the input DMAs fill;
            # transpose-mode doesn't count as PE activity so these must be
            # real matmuls.  One matmul each on a/ut primes PE's sync-wait.
            warm_t = pst_pool.tile([P, SG, BLK], BF16, name="ps_t", tag="ps_t")
            nc.tensor.transpose(warm_t[:, 0, :P], ident_sb, ident_sb)
            warm = pso_pool.tile([P, 512], F32, tag="ps_o")
            for _ in range(26):
                nc.tensor.matmul(warm[:, :P], ident_sb, ident_sb,
                                 start=True, stop=True)
            nc.tensor.matmul(warm[:R, :P], a_sb[:, :R], a_sb[:, :P],
                             start=True, stop=True)
            nc.tensor.matmul(warm[:, :512], ut_sb[:, :P], ut_sb[:, :512],
                             start=True, stop=True)

            # prefetch the remaining block inputs (one 2 MB DMA each)
            for b in range(1, N_BLKS):
                xb = x_pool.tile([P, J, D], BF16, tag="xb")
                xbs.append(xb)
                nc.sync.dma_start(xb, xd[b])

            def back_units(b, pt):
                """yield per-(j,c) update+subtract callables; the final
                block streams its output in quarter pieces to cut the tail"""
                xb = xbs[b]
                last = b == N_BLKS - 1
                pol_tab = _POL_LAST if last else _POL
                pair = [None]

                def unit(j, c):
                    ps_o = pso_pool.tile([P, 512], F32, tag="ps_o")
                    nc.tensor.matmul(
                        ps_o,
                        pt[:, j * P:(j + 1) * P],
                        ut_sb[:, c * 512:(c + 1) * 512],
                        start=True,
                        stop=True,
                    )
                    dst = xb[:, j, c * 512:(c + 1) * 512]
                    pol = pol_tab[j * UPD_CHUNKS + c]
                    if pol == _D:
                        nc.vector.tensor_sub(dst, dst, ps_o)
                    elif pol == _AG:
                        t = tmp_pool.tile([P, 512], BF16, tag="tmp")
                        nc.scalar.copy(t, ps_o)
                        nc.gpsimd.tensor_sub(dst, dst, t)
                    elif pol == _AP1:
                        t2 = tmp_pool.tile([P, 1024], BF16, tag="tmpb")
                        nc.scalar.copy(t2[:, :512], ps_o)
                        pair[0] = t2
                    else:  # _AP2: finish the pair with one 1024-wide sub
                        t2 = pair[0]
                        nc.scalar.copy(t2[:, 512:], ps_o)
                        nc.vector.tensor_sub(
                            xb[:, j, (c - 1) * 512:(c + 1) * 512],
                            xb[:, j, (c - 1) * 512:(c + 1) * 512],
                            t2,
                        )
                    if last:
                        if (c + 1) % 2 == 0:
                            lo, hi = (c - 1) * 512, (c + 1) * 512
                            nc.gpsimd.dma_start(od[b, :, j, lo:hi],
                                                xb[:, j, lo:hi])
                    elif c == UPD_CHUNKS - 1 and j == J - 1:
                        nc.gpsimd.dma_start(od[b], xb)

                for j in range(J):
                    for c in range(UPD_CHUNKS):
                        yield lambda j=j, c=c: unit(j, c)

            def front_units(b):
                """Super-group pipeline: 16 transposes land a [128,8,256]
                bf16 PSUM strip, one ACT copy (read as f32 pairs) moves it
                to SBUF, and the 8 proj matmuls of the PREVIOUS super-group
                run under that copy's latency.  pt lands in pts[b]."""
                ps_p = psp_pool.tile([R, BLK], F32, tag="ps_p")
                xts = [None] * N_SG

                def transposes(sg):
                    ps_t = pst_pool.tile([P, SG, BLK], BF16,
                                         name="ps_t", tag="ps_t")
                    for i in range(SG):
                        k = SG * sg + i
                        for j in range(J):
                            nc.tensor.transpose(
                                ps_t[:, i, j * P:(j + 1) * P],
                                xbs[b][:, j, k * P:(k + 1) * P],
                                ident_sb,
                            )
                    xt_g = xt_pool.tile([P, SG, BLK], BF16, tag="xt_g")
                    nc.scalar.copy(xt_g.bitcast(F32), ps_t.bitcast(F32))
                    xts[sg] = xt_g

                def proj(sg):
                    for i in range(SG):
                        k = SG * sg + i
                        nc.tensor.matmul(
                            ps_p,
                            a_sb[:, k * R:(k + 1) * R],
                            xts[sg][:, i],
                            start=(k == 0),
                            stop=(k == D_CHUNKS - 1),
                        )

                def finish():
                    pt = pt_pool.tile([R, BLK], BF16, tag="pt")
                    nc.vector.tensor_copy(pt, ps_p)
                    pts[b] = pt

                yield lambda: transposes(0)
                for sg in range(1, N_SG):
                    yield lambda sg=sg: transposes(sg)
                    yield lambda sg=sg: proj(sg - 1)
                yield lambda: proj(N_SG - 1)
                yield lambda: finish()

            def drain(it):
                for f in it:
                    f()

            pts = {}
            drain(front_units(0))
            for b in range(1, N_BLKS):
                fu = list(front_units(b))       # 9 units
                bu = list(back_units(b - 1, pts[b - 1]))  # 16 units
                # interleave: 2 back units after each front unit until spent
                order = []
                bi = 0
                for f in fu:
                    order.append(f)
                    for _ in range(2):
                        if bi < len(bu):
                            order.append(bu[bi])
                            bi += 1
                while bi < len(bu):
                    order.append(bu[bi])
                    bi += 1
                drain(order)
            drain(back_units(N_BLKS - 1, pts[N_BLKS - 1]))

    nc.compile()
    return nc


def _get_program():
    if "nc" not in _CACHE:
        _CACHE["nc"] = _build_program()
    return _CACHE["nc"]


def kernel(input, hra_u, **run_kwargs):
    input = np.asarray(input, dtype=np.float32)
    hra_u = np.asarray(hra_u, dtype=np.float32)

    A, UT = _householder_wy(hra_u)
    # pack A [D, R] so partition p holds A[c*128+p, :] at free offset c*R
    a_packed = np.ascontiguousarray(
        A.reshape(D_CHUNKS, P, R).transpose(1, 0, 2).reshape(P, D_CHUNKS * R)
    ).astype(NP_BF16)
    ut_b = UT.astype(NP_BF16)
    ident = np.eye(P, dtype=np.float32).astype(NP_BF16)

    x_flat = np.ascontiguousarray(input.reshape(ROWS, D)).astype(NP_BF16)
    in_maps = [
        {
            "x": x_flat[c * ROWS_PER_CORE:(c + 1) * ROWS_PER_CORE],
            "a": a_packed,
            "ut": ut_b,
            "ident": ident,
        }
        for c in range(N_CORES)
    ]

    nc = _get_program()
    res = run_bass_kernel_spmd(nc, in_maps, core_ids=list(range(N_CORES)),
                               **run_kwargs)
    out = np.concatenate([r["out"] for r in res.results], axis=0)
    if run_kwargs:
        kernel.last_results = res
    return out.astype(np.float32).reshape(B, S, D)
